# revision 54
# baseline (speedup 1.0000x reference)
"""ColorConstancy (multi-scale retinex) Trainium2 kernel.

Full-input contract: kernel(**inputs) takes the unsharded inputs from
setup_inputs() and returns the full (16, 3, 512, 512) float32 output.

Strategy (pure data parallel, batch sharded across 8 cores; 6 planes/core):
  log_img = ln(x + 1e-8)
  illum   = sum_s w_s * gauss2d_s(log_img)        (sigmas 2, 4, 8)
  refl    = log_img - illum
  out     = clip(exp((refl - mean) / (std_ddof1 + 1e-8)), 0, 1)

The 2-D Gaussian is separable: gauss2d_s(X) = U_s @ X @ U_s with U_s the
banded symmetric Toeplitz matrix of the 1-D kernel. Pass 1 computes
A_s = L^T V_s on the TensorEngine (banded: moving dim 128+2c cols per
matmul); the PSUM->SBUF evacuation negates (A_n = -A), split between ACT
and DVE by tuned per-mb patterns (GPSIMD/Pool has no PSUM port, and its
software ops are ~50x the cost-model estimate on HW, so it stays idle).
Pass 2 accumulates psi = I@L - sum_s A_n_s^T V_s = L - illum = refl
directly in PSUM: an fp32r identity matmul supplies +L at full PE rate
(moving dim 512 >= 256) and full precision - the dark-pixel tail |L|~18
cannot afford fp16's 8e-3 ulp there. Pass 1 instead reads L as the high
halves of the fp32 log via a bf16 bitcast view (free truncation; blur
averaging keeps that noise ~2e-3, and 2-byte weight loads run full rate).
refl stays fp32 (same tail argument). Stats: per-mb DVE bn_stats +
bn_aggr, then a ones-matmul sums/broadcasts across partitions into a PSUM
slot borrowed from the psi ring. Output: one fused Exp activation
(bias/scale = normalization), a 4x-mode bf16 min(.,1), bf16 DMA out
(fp16 would go subnormal below 6e-5 and fail the checker's rel-err floor).

HW-measured constraints honored here: table-based Ln must be issued in
<=512-col chunks ([P,2048] Ln is ~2.5x slower; Exp is fine one-shot), and
fp8 DoubleRow pass 2 gave no speedup (PE is not the HW critical path) at
2e-2-threatening precision. x is fp16 (host-cast), y bf16.
PSUM: 2+2(x2-bank)+2 banks, everything double-buffered.
"""

import numpy as np

N_CORES = 8
NPLANES = 6          # 2 batch images x 3 channels per core
H = W = 512
P = 128
NB = H // P          # 4 row blocks
CS = (6, 12, 24)     # band half-widths for sigma 2, 4, 8 (K = 13, 25, 49)
EPS = 1e-8
NPIX = H * W

_PROGRAM_CACHE = {}

# --- tuning knobs (per-mb engine patterns: 'A'=ACT, 'D'=DVE) ---
EV_A2 = "AAAA"       # pass-1 sigma2 evac engine per mb
EV_A4 = "AAAA"       # pass-1 sigma4 evac (ignored when MERGE_A48)
EV_A8 = "DDDD"       # pass-1 sigma8 evac (merged a48 evac when MERGE_A48)
EV_RF = "AAAD"       # pass-2 psi->refl evac
# gpsimd/Pool software ops measured ~50x their cost-model estimate on HW
# (ucode library load per ISA call?) - keep everything off Pool.
FP8_PASS2 = False    # fp8 pass2: no HW speedup (PE not critical) and rel
#                      err 1.6e-2 is too close to the 2e-2 gate
MERGE_A48 = True     # sigma4+sigma8 share one 2-bank PSUM tile + one evac
USE_LAYERNORM = False  # gpsimd layernorm: F<=32 only, unusable for 2048
MIN_ON_POOL = False  # final min(yt,1) on Pool
TAIL_ON_POOL = False  # stats tail scalar ops on Pool
USE_ALLREDUCE = False  # gpsimd partition_all_reduce vs ones-matmul broadcast
# stride-2 stats looked tempting (halves bn_stats) but the sigma error
# ~0.002 is amplified by |z|~17 at the distribution tails: 3e-2 rel err.
STATS_STRIDE = 1     # bn_stats column stride (must stay 1)
SBUF_BUFS = 2
DEBUG_OUT = None     # dev only: "refl" dumps refl instead of the output


def _ncol(kb, c):
    """Output column range that input row block kb touches through a band-c kernel."""
    return max(0, P * kb - c), min(W, P * (kb + 1) + c)


def build_program(reps=1, ablate=()):
    """Build + compile the per-core Bass program. reps>1 wraps the whole
    computation in a hardware loop (for timing by subtraction)."""
    ablate = set(ablate)
    import concourse.bacc as bacc
    import concourse.tile as tile
    from concourse import mybir, bass_isa

    f32 = mybir.dt.float32
    f16 = mybir.dt.float16
    bf16 = mybir.dt.bfloat16
    f8 = mybir.dt.float8e4
    AF = mybir.ActivationFunctionType
    DR = mybir.MatmulPerfMode.DoubleRow

    # The activation-table chooser picks the first set containing each
    # function, which puts Ln in "natural_log" and Exp in "exp_and_others" and
    # reloads tables (~2.7us each) every plane. Narrow the cached table map so
    # only the combined "natural_log_exp_and_others" set provides Ln/Exp; then
    # one load serves the whole kernel.
    from concourse.hw_specs import get_activation_tables
    _tabs = get_activation_tables("gen3")
    for _name, _fset in _tabs.items():
        if _name != "natural_log_exp_and_others":
            _fset.discard(AF.Ln)
            _fset.discard(AF.Exp)

    f32r = mybir.dt.float32r
    nc = bacc.Bacc("TRN2", target_bir_lowering=False, debug=False,
                   num_devices=N_CORES)
    x = nc.declare_dram_parameter("x", [NPLANES, H, W], f16, isOutput=False)
    vs = [nc.declare_dram_parameter(f"v{s}", [H, W], f16, isOutput=False)
          for s in range(3)]
    vq = [nc.declare_dram_parameter(f"vq{i}", [H, W], f8, isOutput=False)
          for i in range(2)] if FP8_PASS2 else []
    ident = nc.declare_dram_parameter("ident", [P, P], f32r, isOutput=False)
    # bf16 output: the dark tail produces y ~ 4e-8, subnormal in fp16
    # (min normal 6e-5); the subnormal step vs the checker's 1e-6 rel floor
    # costs 3e-2 rel err. bf16 keeps tiny values normal (worst 0.4% ulp).
    y = nc.declare_dram_parameter("y", [NPLANES, H, W], bf16, isOutput=True)

    with tile.TileContext(nc) as tc:
        with (
            tc.tile_pool(name="consts", bufs=1) as consts,
            tc.tile_pool(name="xin", bufs=SBUF_BUFS) as xpool,
            tc.tile_pool(name="logp", bufs=SBUF_BUFS) as lpool,
            tc.tile_pool(name="apool", bufs=SBUF_BUFS) as apool,
            tc.tile_pool(name="refl", bufs=SBUF_BUFS) as rpool,
            tc.tile_pool(name="yout", bufs=SBUF_BUFS) as ypool,
            tc.tile_pool(name="small", bufs=2) as spool,
            tc.tile_pool(name="ps2p", bufs=2, space="PSUM") as ps2p,
            tc.tile_pool(name="ps48p", bufs=2, space="PSUM") as ps48p,
            tc.tile_pool(name="psip", bufs=2, space="PSUM") as psip,
        ):
            # Plane 0's input DMA is issued FIRST: the DMA queue is
            # serial, so loading it ahead of the ~5us of V constants lets
            # Ln/pass-1 of plane 0 overlap the remaining const loads
            # (single-shot execution starts ~4us earlier).
            xt0 = xpool.tile([P, NB, W], f16, tag="x", name="xt0")
            nc.sync.dma_start(
                out=xt0, in_=x[0].rearrange("(kb q) w -> q kb w", q=P))
            # Banded blur matrices, resident for the whole kernel.
            # Layout [p, kb, n]: matrix row = kb*128 + p.
            V16 = []
            for s in range(3):
                vt = consts.tile([P, NB, W], f16, tag=f"v{s}")
                nc.sync.dma_start(
                    out=vt, in_=vs[s].rearrange("(kb p) n -> p kb n", p=P))
                V16.append(vt)
            V8Q = []
            for i, dram in enumerate(vq):
                vtq = consts.tile([P, NB, W], f8, tag=f"vq{i}")
                nc.sync.dma_start(
                    out=vtq, in_=dram.rearrange("(kb p) n -> p kb n", p=P))
                V8Q.append(vtq)
            i16 = consts.tile([P, P], f32r, tag="ident")
            nc.sync.dma_start(out=i16, in_=ident[:, :])
            epst = consts.tile([P, 1], f32, tag="eps")
            nc.vector.memset(epst, EPS)
            # memset 1.0 (not 1/P): the stats matmul must produce raw
            # partition sums, matching what partition_all_reduce would give
            # (the tail divides by P itself).
            ones16 = consts.tile([P, P], f16, tag="ones16")
            nc.vector.memset(ones16, 1.0)

            tail_eng = nc.gpsimd if TAIL_ON_POOL else nc.vector

            def evac(eng, out, in_, negate):
                """PSUM -> SBUF evacuation on engine 'A'(CT) or 'D'(VE)."""
                if "evac" in ablate:
                    return
                if eng == "A":
                    nc.scalar.activation(out=out, in_=in_, func=AF.Copy,
                                         scale=-1.0 if negate else 1.0)
                else:
                    if negate:
                        nc.vector.tensor_scalar_mul(out=out, in0=in_,
                                                    scalar1=-1.0)
                    else:
                        nc.vector.tensor_copy(out=out, in_=in_)

            def emit_planes():
                state = {}
                state2 = {}

                def front(p):
                    # load -> ln(fp32r full precision: the dark-pixel tail
                    # |L|~18 would lose 8e-3 abs in fp16, which the +L
                    # identity matmul would pass straight into refl) ->
                    # fp16 copy for pass 1 -> pass 1 (A_n_s = -L^T V_s)
                    if p == 0 and not hasattr(front, "_used_xt0"):
                        front._used_xt0 = True
                        xt = xt0
                    else:
                        xt = xpool.tile([P, NB, W], f16, tag="x")
                        if "dma" in ablate:
                            nc.vector.memset(xt, 0.5)
                        else:
                            nc.sync.dma_start(
                                out=xt,
                                in_=x[p].rearrange("(kb q) w -> q kb w", q=P))
                    lt32 = lpool.tile([P, NB, W], f32r, tag="l32")
                    if "act" in ablate:
                        nc.scalar.copy(out=lt32, in_=xt)
                    else:
                        # HW: table-based Ln must stay in <=512-col chunks
                        # ([P,1024]/[P,2048] Ln measured ~2.5x slower/plane)
                        for mb in range(NB):
                            nc.scalar.activation(
                                out=lt32[:, mb, :], in_=xt[:, mb, :],
                                func=AF.Exp if "acttab" in ablate else AF.Ln,
                                bias=epst, scale=1.0)
                    # pass 1 reads the high halves of lt32 as bf16 weights
                    # (free truncation; 2-byte weight loads run full rate on
                    # HW, unlike 4-byte). Blur averaging keeps the bf16
                    # noise ~2e-3; the precision-critical +L path still
                    # reads full fp32r.
                    lt = lt32.bitcast(bf16)[:, :, 1::2]

                    adt = (f16, f8, f8) if FP8_PASS2 else (f16, f16, f16)
                    if "acttab" in ablate:
                        # table probe: Ln emitted as Exp (same ACT table)
                        pass
                    if MERGE_A48:
                        a2n = apool.tile([P, NB, W], f16, tag="a2")
                        a48n = apool.tile([P, 2, NB, W], adt[1], tag="a48")
                        An = (a2n, a48n)
                    else:
                        An = [apool.tile([P, NB, W], adt[s], tag=f"a{s}",
                                         name=f"a16_{s}")
                              for s in range(3)]
                    if "evac" in ablate:
                        for a in An:
                            nc.vector.memset(a, 0.5)
                    for mb in range(NB):
                        ps2 = ps2p.tile([P, W], f32, tag="ps")
                        ps48 = ps48p.tile([P, 2, W], f32, tag="ps")
                        psv = (ps2, ps48[:, 0, :], ps48[:, 1, :])
                        if "pe" not in ablate:
                            for s in range(3):
                                for kb in range(NB):
                                    lo, hi = _ncol(kb, CS[s])
                                    nc.tensor.matmul(
                                        psv[s][:, lo:hi],
                                        lt[:, kb, P * mb:P * (mb + 1)],
                                        V16[s][:, kb, lo:hi],
                                        start=(kb == 0), stop=(kb == NB - 1),
                                    )
                        if MERGE_A48:
                            evac(EV_A2[mb], An[0][:, mb, :], ps2, negate=True)
                            evac(EV_A8[mb], An[1][:, :, mb, :], ps48,
                                 negate=True)
                        else:
                            evpat = (EV_A2, EV_A4, EV_A8)
                            for s in range(3):
                                evac(evpat[s][mb], An[s][:, mb, :], psv[s],
                                     negate=True)
                    state[p] = (lt32, An)

                def back(p):
                    # pass 2: psi = L - illum in PSUM -> refl -> norm -> out
                    lt32, An = state.pop(p)
                    # refl must be fp32: its dark tail |refl|~18 would lose
                    # 8e-3 abs in fp16, directly visible in the output
                    rt = rpool.tile([P, NB, W], f32, tag="r")
                    if "evac" in ablate:
                        nc.vector.memset(rt, 0.1)
                    if not USE_LAYERNORM:
                        st6 = spool.tile([P, NB, 6], f32, tag="st6")
                    for mb in range(NB):
                        psi = psip.tile([P, W], f32, tag="psi")
                        if "pe" not in ablate:
                            # fp32r identity matmul: full rate (moving dim
                            # 512 >= 256) with ~tf32 precision for +L
                            nc.tensor.matmul(
                                psi, i16, lt32[:, mb, :],
                                start=True, stop=False)
                            # sigma2 fp16, kb-granular
                            for kb in range(NB):
                                lo, hi = _ncol(kb, CS[0])
                                nc.tensor.matmul(
                                    psi[:, lo:hi],
                                    An[0][:, kb, P * mb:P * (mb + 1)],
                                    V16[0][:, kb, lo:hi],
                                    start=False, stop=False,
                                )
                            if FP8_PASS2:
                                # sigma4/8 fp8 DoubleRow: 256-row kb-pairs
                                for si, s in enumerate((1, 2)):
                                    c = CS[s]
                                    if MERGE_A48:
                                        av = An[1][:, si]
                                    else:
                                        av = An[s]
                                    for kp in range(NB // 2):
                                        lo = max(0, 2 * P * kp - c)
                                        hi = min(W, 2 * P * (kp + 1) + c)
                                        nc.tensor.matmul(
                                            psi[:, lo:hi],
                                            av[:, 2 * kp:2 * kp + 2,
                                               P * mb:P * (mb + 1)],
                                            V8Q[si][:, 2 * kp:2 * kp + 2,
                                                    lo:hi],
                                            start=False,
                                            stop=(s == 2 and kp == 1),
                                            perf_mode=DR,
                                        )
                            else:
                                for s in (1, 2):
                                    av = An[1][:, s - 1] if MERGE_A48 \
                                        else An[s]
                                    for kb in range(NB):
                                        lo, hi = _ncol(kb, CS[s])
                                        nc.tensor.matmul(
                                            psi[:, lo:hi],
                                            av[:, kb, P * mb:P * (mb + 1)],
                                            V16[s][:, kb, lo:hi],
                                            start=False,
                                            stop=(s == 2 and kb == NB - 1),
                                        )
                        else:
                            nc.vector.memset(psi, 0.3)
                        evac(EV_RF[mb], rt[:, mb, :], psi, negate=False)
                        if not USE_LAYERNORM:
                            nc.vector.bn_stats(out=st6[:, mb, :],
                                               in_=rt[:, mb, ::STATS_STRIDE])

                    # plane-wide mean/var: per-partition bn stats aggregated
                    mv = spool.tile([P, 2], f32, tag="mv")
                    nc.vector.bn_aggr(out=mv, in_=st6)
                    t2 = spool.tile([P, 2], f16, tag="t2")
                    # E[x^2]_p = mean_p*mean_p + var_p, one fused op
                    nc.vector.scalar_tensor_tensor(
                        out=t2[:, 1:2], in0=mv[:, 0:1],
                        scalar=mv[:, 0:1], in1=mv[:, 1:2],
                        op0=mybir.AluOpType.mult,
                        op1=mybir.AluOpType.add)
                    nc.vector.tensor_copy(out=t2[:, 0:1], in_=mv[:, 0:1])
                    state2[p] = (rt, t2)

                def back_tail(p):
                    # deferred one pipeline step: by the time psS issues, the
                    # in-order PE stream has already run front(p+2)'s
                    # matmuls, so PE does not stall waiting for t2
                    rt, t2 = state2.pop(p)
                    yt = ypool.tile([P, NB, W], bf16, tag="y")
                    if "taildep" in ablate:
                        # timing probe: break Exp's dependency on the stats
                        # tail (constant rs/nbv written at plane start)
                        fin2 = spool.tile([P, 2], f32, tag="fin2")
                        nc.vector.memset(fin2, 1.0)
                    # ones-weights matmul sums across partitions AND
                    # broadcasts; out slot borrowed from the psi ring
                    # (same tag+shape: no extra PSUM bank).
                    psS = psip.tile([P, W], f32, tag="psi", name="psS")
                    ar = psS[:, 0:2]
                    nc.tensor.matmul(ar, ones16, t2, start=True, stop=True)

                    # short critical path to rs/nbv (every hop here delays
                    # the output Exp by an HW sem roundtrip):
                    #   fin01 = ar/P -> nvar = mean^2-E[x^2] (fused)
                    #   rs = exp(-0.5*ln(-c*nvar))  [ddof-1 c in the
                    #   negative Ln scale; drops the +eps (1e-8 rel on
                    #   std~1) and the reciprocal table op entirely]
                    fin = spool.tile([P, 4], f32, tag="fin")
                    mean = fin[:, 0:1]
                    nvar = fin[:, 1:2]
                    rs = fin[:, 2:3]
                    nbv = fin[:, 3:4]
                    nc.vector.tensor_scalar_mul(out=fin[:, 0:2],
                                                in0=ar[:, 0:2],
                                                scalar1=1.0 / P)
                    nc.vector.scalar_tensor_tensor(
                        out=nvar, in0=mean, scalar=mean, in1=fin[:, 1:2],
                        op0=mybir.AluOpType.mult,
                        op1=mybir.AluOpType.subtract)
                    npix = NPIX // STATS_STRIDE
                    nc.scalar.activation(out=rs, in_=nvar, func=AF.Ln,
                                         scale=-float(npix) / (npix - 1))
                    nc.scalar.activation(out=rs, in_=rs, func=AF.Exp,
                                         scale=-0.5)
                    # nbv = -mean * rs (one fused tensor_scalar: two ops)
                    nc.vector.tensor_scalar(out=nbv, in0=mean, scalar1=rs,
                                            scalar2=-1.0,
                                            op0=mybir.AluOpType.mult,
                                            op1=mybir.AluOpType.mult)
                    if "taildep" in ablate:
                        rs, nbv = fin2[:, 0:1], fin2[:, 1:2]
                    ydst = y[p].rearrange("(kb q) w -> q kb w", q=P)
                    if "act" in ablate:
                        nc.scalar.copy(out=yt, in_=rt)
                    else:
                        nc.scalar.activation(out=yt, in_=rt, func=AF.Exp,
                                             bias=nbv, scale=rs)
                    if MIN_ON_POOL:
                        nc.gpsimd.tensor_scalar_min(out=yt, in0=yt,
                                                    scalar1=1.0)
                    else:
                        nc.vector.tensor_scalar_min(out=yt, in0=yt,
                                                    scalar1=1.0)
                    if DEBUG_OUT == "refl":
                        nc.vector.tensor_copy(out=yt, in_=rt)
                    if "dma" not in ablate:
                        nc.sync.dma_start(out=ydst, in_=yt)

                # software-pipelined: pass 1 of plane p overlaps pass 2 of
                # p-1; the stats tail of p-2 slots between them
                for p in range(NPLANES + 2):
                    if p < NPLANES:
                        front(p)
                    if p >= 2:
                        back_tail(p - 2)
                    if 1 <= p <= NPLANES:
                        back(p - 1)

            if isinstance(reps, str) and reps.startswith("u"):
                for _ in range(int(reps[1:])):
                    emit_planes()
            elif reps == 1:
                emit_planes()
            else:
                from concourse import mybir as _mb
                with tc.For_i(0, reps, 1,
                              hint_engines=(_mb.EngineType.PE,)):
                    emit_planes()

    nc.compile()
    return nc


def get_program(reps=1):
    if reps not in _PROGRAM_CACHE:
        _PROGRAM_CACHE[reps] = build_program(reps)
    return _PROGRAM_CACHE[reps]


def _u_factors(k0, k1, k2):
    w = np.array([1.0, 0.75, 0.5], dtype=np.float64)
    w /= w.sum()
    us = []
    for s, k2d in enumerate((k0, k1, k2)):
        g = np.asarray(k2d)[0, 0].astype(np.float64)
        us.append((g.sum(axis=0), np.sqrt(w[s])))
    return us


def _toeplitz(u, scale, dtype, feedback=False):
    """Banded symmetric Toeplitz of scale*u. With feedback=True, quantize
    with per-column running error compensation (kills net kernel-mass bias
    from coarse dtypes like fp8)."""
    c = len(u) // 2
    V64 = np.zeros((H, W), dtype=np.float64)
    for d in range(-c, c + 1):
        V64 += np.diag(np.full(H - abs(d), scale * u[c + d]), k=d)
    if not feedback:
        return V64.astype(dtype)
    Vq = np.zeros((H, W), dtype=dtype)
    for j in range(W):
        lo, hi = max(0, j - c), min(H, j + c + 1)
        carry = 0.0
        for i in range(lo, hi):
            q = np.asarray(V64[i, j] + carry, dtype=dtype)
            carry = (V64[i, j] + carry) - float(q)
            Vq[i, j] = q
    return Vq


def build_v_matrices(k0, k1, k2):
    """fp16 banded Toeplitz matrices sqrt(w_s) * toeplitz(u_s) from the
    reference's 2-D depthwise kernels (u_s = column sums of the normalized
    2-D kernel, exact by separability)."""
    return [_toeplitz(u, sc, np.float16) for u, sc in _u_factors(k0, k1, k2)]


def build_const_inputs(k0, k1, k2):
    """Per-core constant input tensors (same on every core)."""
    us = _u_factors(k0, k1, k2)
    m = {f"v{s}": _toeplitz(u, sc, np.float16)
         for s, (u, sc) in enumerate(us)}
    if FP8_PASS2:
        import ml_dtypes
        for i, s in enumerate((1, 2)):
            u, sc = us[s]
            m[f"vq{i}"] = _toeplitz(u, sc, ml_dtypes.float8_e4m3,
                                    feedback=True)
    m["ident"] = np.eye(P, dtype=np.float32)
    return m


def kernel(rgb_image, k0, k1, k2):
    from concourse.bass_utils import run_bass_kernel_spmd

    nc = get_program()
    consts = build_const_inputs(k0, k1, k2)
    xs = np.asarray(rgb_image, dtype=np.float16)
    B = xs.shape[0]
    per_core = B // N_CORES
    in_maps = []
    for c in range(N_CORES):
        m = {"x": np.ascontiguousarray(
            xs[c * per_core:(c + 1) * per_core].reshape(NPLANES, H, W))}
        m.update(consts)
        in_maps.append(m)
    res = run_bass_kernel_spmd(nc, in_maps, list(range(N_CORES)))
    out = np.empty((B, 3, H, W), dtype=np.float32)
    for c in range(N_CORES):
        out[c * per_core:(c + 1) * per_core] = (
            res.results[c]["y"].astype(np.float32).reshape(per_core, 3, H, W))
    return out


# revision 55
# speedup vs baseline: 1.0076x; 1.0076x over previous
"""ColorConstancy (multi-scale retinex) Trainium2 kernel.

Full-input contract: kernel(**inputs) takes the unsharded inputs from
setup_inputs() and returns the full (16, 3, 512, 512) float32 output.

Strategy (pure data parallel, batch sharded across 8 cores; 6 planes/core):
  log_img = ln(x + 1e-8)
  illum   = sum_s w_s * gauss2d_s(log_img)        (sigmas 2, 4, 8)
  refl    = log_img - illum
  out     = clip(exp((refl - mean) / (std_ddof1 + 1e-8)), 0, 1)

The 2-D Gaussian is separable: gauss2d_s(X) = U_s @ X @ U_s with U_s the
banded symmetric Toeplitz matrix of the 1-D kernel. Pass 1 computes
A_s = L^T V_s on the TensorEngine (banded: moving dim 128+2c cols per
matmul); the PSUM->SBUF evacuation negates (A_n = -A), split between ACT
and DVE by tuned per-mb patterns (GPSIMD/Pool has no PSUM port, and its
software ops are ~50x the cost-model estimate on HW, so it stays idle).
Pass 2 accumulates psi = I@L - sum_s A_n_s^T V_s = L - illum = refl
directly in PSUM: an fp32r identity matmul supplies +L at full PE rate
(moving dim 512 >= 256) and full precision - the dark-pixel tail |L|~18
cannot afford fp16's 8e-3 ulp there. Pass 1 instead reads L as the high
halves of the fp32 log via a bf16 bitcast view (free truncation; blur
averaging keeps that noise ~2e-3, and 2-byte weight loads run full rate).
refl stays fp32 (same tail argument). Stats: per-mb DVE bn_stats +
bn_aggr, then a ones-matmul sums/broadcasts across partitions into a PSUM
slot borrowed from the psi ring. Output: one fused Exp activation
(bias/scale = normalization), a 4x-mode bf16 min(.,1), bf16 DMA out
(fp16 would go subnormal below 6e-5 and fail the checker's rel-err floor).

HW-measured constraints honored here: table-based Ln must be issued in
<=512-col chunks ([P,2048] Ln is ~2.5x slower; Exp is fine one-shot), and
fp8 DoubleRow pass 2 gave no speedup (PE is not the HW critical path) at
2e-2-threatening precision. x is fp16 (host-cast), y bf16.
PSUM: 2+2(x2-bank)+2 banks, everything double-buffered.
"""

import numpy as np

N_CORES = 8
NPLANES = 6          # 2 batch images x 3 channels per core
H = W = 512
P = 128
NB = H // P          # 4 row blocks
CS = (6, 12, 24)     # band half-widths for sigma 2, 4, 8 (K = 13, 25, 49)
EPS = 1e-8
NPIX = H * W

_PROGRAM_CACHE = {}

# --- tuning knobs (per-mb engine patterns: 'A'=ACT, 'D'=DVE) ---
EV_A2 = "AAAA"       # pass-1 sigma2 evac engine per mb
EV_A4 = "AAAA"       # pass-1 sigma4 evac (ignored when MERGE_A48)
EV_A8 = "DDDD"       # pass-1 sigma8 evac (merged a48 evac when MERGE_A48)
EV_RF = "AAAD"       # pass-2 psi->refl evac
# gpsimd/Pool software ops measured ~50x their cost-model estimate on HW
# (ucode library load per ISA call?) - keep everything off Pool.
FP8_PASS2 = False    # fp8 pass2: no HW speedup (PE not critical) and rel
#                      err 1.6e-2 is too close to the 2e-2 gate
MERGE_A48 = True     # sigma4+sigma8 share one 2-bank PSUM tile + one evac
USE_LAYERNORM = False  # gpsimd layernorm: F<=32 only, unusable for 2048
MIN_ON_POOL = False  # final min(yt,1) on Pool
TAIL_ON_POOL = False  # stats tail scalar ops on Pool
USE_ALLREDUCE = False  # gpsimd partition_all_reduce vs ones-matmul broadcast
# stride-2 stats looked tempting (halves bn_stats) but the sigma error
# ~0.002 is amplified by |z|~17 at the distribution tails: 3e-2 rel err.
STATS_STRIDE = 1     # bn_stats column stride (must stay 1)
SBUF_BUFS = 2
DEBUG_OUT = None     # dev only: "refl" dumps refl instead of the output


def _ncol(kb, c):
    """Output column range that input row block kb touches through a band-c kernel."""
    return max(0, P * kb - c), min(W, P * (kb + 1) + c)


def build_program(reps=1, ablate=()):
    """Build + compile the per-core Bass program. reps>1 wraps the whole
    computation in a hardware loop (for timing by subtraction)."""
    ablate = set(ablate)
    import concourse.bacc as bacc
    import concourse.tile as tile
    from concourse import mybir, bass_isa

    f32 = mybir.dt.float32
    f16 = mybir.dt.float16
    bf16 = mybir.dt.bfloat16
    f8 = mybir.dt.float8e4
    AF = mybir.ActivationFunctionType
    DR = mybir.MatmulPerfMode.DoubleRow

    # The activation-table chooser picks the first set containing each
    # function, which puts Ln in "natural_log" and Exp in "exp_and_others" and
    # reloads tables (~2.7us each) every plane. Narrow the cached table map so
    # only the combined "natural_log_exp_and_others" set provides Ln/Exp; then
    # one load serves the whole kernel.
    from concourse.hw_specs import get_activation_tables
    _tabs = get_activation_tables("gen3")
    for _name, _fset in _tabs.items():
        if _name != "natural_log_exp_and_others":
            _fset.discard(AF.Ln)
            _fset.discard(AF.Exp)

    f32r = mybir.dt.float32r
    nc = bacc.Bacc("TRN2", target_bir_lowering=False, debug=False,
                   num_devices=N_CORES)
    x = nc.declare_dram_parameter("x", [NPLANES, H, W], f16, isOutput=False)
    vs = [nc.declare_dram_parameter(f"v{s}", [H, W], f16, isOutput=False)
          for s in range(3)]
    vq = [nc.declare_dram_parameter(f"vq{i}", [H, W], f8, isOutput=False)
          for i in range(2)] if FP8_PASS2 else []
    ident = nc.declare_dram_parameter("ident", [P, P], f32r, isOutput=False)
    # bf16 output: the dark tail produces y ~ 4e-8, subnormal in fp16
    # (min normal 6e-5); the subnormal step vs the checker's 1e-6 rel floor
    # costs 3e-2 rel err. bf16 keeps tiny values normal (worst 0.4% ulp).
    y = nc.declare_dram_parameter("y", [NPLANES, H, W], bf16, isOutput=True)

    with tile.TileContext(nc) as tc:
        with (
            tc.tile_pool(name="consts", bufs=1) as consts,
            tc.tile_pool(name="xin", bufs=SBUF_BUFS) as xpool,
            tc.tile_pool(name="logp", bufs=SBUF_BUFS) as lpool,
            tc.tile_pool(name="apool", bufs=SBUF_BUFS) as apool,
            tc.tile_pool(name="refl", bufs=SBUF_BUFS) as rpool,
            tc.tile_pool(name="yout", bufs=SBUF_BUFS) as ypool,
            tc.tile_pool(name="small", bufs=2) as spool,
            tc.tile_pool(name="ps2p", bufs=2, space="PSUM") as ps2p,
            tc.tile_pool(name="ps48p", bufs=2, space="PSUM") as ps48p,
            tc.tile_pool(name="psip", bufs=2, space="PSUM") as psip,
        ):
            # Plane 0's input DMA is issued FIRST: the DMA queue is
            # serial, so loading it ahead of the ~5us of V constants lets
            # Ln/pass-1 of plane 0 overlap the remaining const loads
            # (single-shot execution starts ~4us earlier).
            xt0 = xpool.tile([P, NB, W], f16, tag="x", name="xt0")
            nc.sync.dma_start(
                out=xt0, in_=x[0].rearrange("(kb q) w -> q kb w", q=P))
            # Banded blur matrices, resident for the whole kernel.
            # Layout [p, kb, n]: matrix row = kb*128 + p.
            V16 = []
            for s in range(3):
                vt = consts.tile([P, NB, W], f16, tag=f"v{s}")
                nc.sync.dma_start(
                    out=vt, in_=vs[s].rearrange("(kb p) n -> p kb n", p=P))
                V16.append(vt)
            V8Q = []
            for i, dram in enumerate(vq):
                vtq = consts.tile([P, NB, W], f8, tag=f"vq{i}")
                nc.sync.dma_start(
                    out=vtq, in_=dram.rearrange("(kb p) n -> p kb n", p=P))
                V8Q.append(vtq)
            i16 = consts.tile([P, P], f32r, tag="ident")
            nc.sync.dma_start(out=i16, in_=ident[:, :])
            epst = consts.tile([P, 1], f32, tag="eps")
            nc.vector.memset(epst, EPS)
            # memset 1.0 (not 1/P): the stats matmul must produce raw
            # partition sums, matching what partition_all_reduce would give
            # (the tail divides by P itself).
            ones16 = consts.tile([P, P], f16, tag="ones16")
            nc.vector.memset(ones16, 1.0)

            tail_eng = nc.gpsimd if TAIL_ON_POOL else nc.vector

            def evac(eng, out, in_, negate):
                """PSUM -> SBUF evacuation on engine 'A'(CT) or 'D'(VE)."""
                if "evac" in ablate:
                    return
                if eng == "A":
                    nc.scalar.activation(out=out, in_=in_, func=AF.Copy,
                                         scale=-1.0 if negate else 1.0)
                else:
                    if negate:
                        nc.vector.tensor_scalar_mul(out=out, in0=in_,
                                                    scalar1=-1.0)
                    else:
                        nc.vector.tensor_copy(out=out, in_=in_)

            def emit_planes():
                state = {}
                state2 = {}

                def front(p):
                    # load -> ln(fp32r full precision: the dark-pixel tail
                    # |L|~18 would lose 8e-3 abs in fp16, which the +L
                    # identity matmul would pass straight into refl) ->
                    # fp16 copy for pass 1 -> pass 1 (A_n_s = -L^T V_s)
                    if p == 0 and not hasattr(front, "_used_xt0"):
                        front._used_xt0 = True
                        xt = xt0
                    else:
                        xt = xpool.tile([P, NB, W], f16, tag="x")
                        if "dma" in ablate:
                            nc.vector.memset(xt, 0.5)
                        else:
                            nc.sync.dma_start(
                                out=xt,
                                in_=x[p].rearrange("(kb q) w -> q kb w", q=P))
                    lt32 = lpool.tile([P, NB, W], f32r, tag="l32")
                    if "act" in ablate:
                        nc.scalar.copy(out=lt32, in_=xt)
                    else:
                        # HW: table-based Ln must stay in <=512-col chunks
                        # ([P,1024]/[P,2048] Ln measured ~2.5x slower/plane)
                        for mb in range(NB):
                            nc.scalar.activation(
                                out=lt32[:, mb, :], in_=xt[:, mb, :],
                                func=AF.Exp if "acttab" in ablate else AF.Ln,
                                bias=epst, scale=1.0)
                    # pass 1 reads the high halves of lt32 as bf16 weights
                    # (free truncation; 2-byte weight loads run full rate on
                    # HW, unlike 4-byte). Blur averaging keeps the bf16
                    # noise ~2e-3; the precision-critical +L path still
                    # reads full fp32r.
                    lt = lt32.bitcast(bf16)[:, :, 1::2]

                    adt = (f16, f8, f8) if FP8_PASS2 else (f16, f16, f16)
                    if "acttab" in ablate:
                        # table probe: Ln emitted as Exp (same ACT table)
                        pass
                    if MERGE_A48:
                        a2n = apool.tile([P, NB, W], f16, tag="a2")
                        a48n = apool.tile([P, 2, NB, W], adt[1], tag="a48")
                        An = (a2n, a48n)
                    else:
                        An = [apool.tile([P, NB, W], adt[s], tag=f"a{s}",
                                         name=f"a16_{s}")
                              for s in range(3)]
                    if "evac" in ablate:
                        for a in An:
                            nc.vector.memset(a, 0.5)
                    for mb in range(NB):
                        ps2 = ps2p.tile([P, W], f32, tag="ps")
                        ps48 = ps48p.tile([P, 2, W], f32, tag="ps")
                        psv = (ps2, ps48[:, 0, :], ps48[:, 1, :])
                        if "pe" not in ablate:
                            for s in range(3):
                                for kb in range(NB):
                                    lo, hi = _ncol(kb, CS[s])
                                    nc.tensor.matmul(
                                        psv[s][:, lo:hi],
                                        lt[:, kb, P * mb:P * (mb + 1)],
                                        V16[s][:, kb, lo:hi],
                                        start=(kb == 0), stop=(kb == NB - 1),
                                    )
                        if MERGE_A48:
                            evac(EV_A2[mb], An[0][:, mb, :], ps2, negate=True)
                            evac(EV_A8[mb], An[1][:, :, mb, :], ps48,
                                 negate=True)
                        else:
                            evpat = (EV_A2, EV_A4, EV_A8)
                            for s in range(3):
                                evac(evpat[s][mb], An[s][:, mb, :], psv[s],
                                     negate=True)
                    state[p] = (lt32, An)

                def back(p):
                    # pass 2: psi = L - illum in PSUM -> refl -> norm -> out
                    lt32, An = state.pop(p)
                    # refl must be fp32: its dark tail |refl|~18 would lose
                    # 8e-3 abs in fp16, directly visible in the output
                    rt = rpool.tile([P, NB, W], f32, tag="r")
                    if "evac" in ablate:
                        nc.vector.memset(rt, 0.1)
                    if not USE_LAYERNORM:
                        st6 = spool.tile([P, NB, 6], f32, tag="st6")
                    for mb in range(NB):
                        psi = psip.tile([P, W], f32, tag="psi")
                        if "pe" not in ablate:
                            # fp32r identity matmul: full rate (moving dim
                            # 512 >= 256) with ~tf32 precision for +L
                            nc.tensor.matmul(
                                psi, i16, lt32[:, mb, :],
                                start=True, stop=False)
                            # sigma2 fp16, kb-granular
                            for kb in range(NB):
                                lo, hi = _ncol(kb, CS[0])
                                nc.tensor.matmul(
                                    psi[:, lo:hi],
                                    An[0][:, kb, P * mb:P * (mb + 1)],
                                    V16[0][:, kb, lo:hi],
                                    start=False, stop=False,
                                )
                            if FP8_PASS2:
                                # sigma4/8 fp8 DoubleRow: 256-row kb-pairs
                                for si, s in enumerate((1, 2)):
                                    c = CS[s]
                                    if MERGE_A48:
                                        av = An[1][:, si]
                                    else:
                                        av = An[s]
                                    for kp in range(NB // 2):
                                        lo = max(0, 2 * P * kp - c)
                                        hi = min(W, 2 * P * (kp + 1) + c)
                                        nc.tensor.matmul(
                                            psi[:, lo:hi],
                                            av[:, 2 * kp:2 * kp + 2,
                                               P * mb:P * (mb + 1)],
                                            V8Q[si][:, 2 * kp:2 * kp + 2,
                                                    lo:hi],
                                            start=False,
                                            stop=(s == 2 and kp == 1),
                                            perf_mode=DR,
                                        )
                            else:
                                for s in (1, 2):
                                    av = An[1][:, s - 1] if MERGE_A48 \
                                        else An[s]
                                    for kb in range(NB):
                                        lo, hi = _ncol(kb, CS[s])
                                        nc.tensor.matmul(
                                            psi[:, lo:hi],
                                            av[:, kb, P * mb:P * (mb + 1)],
                                            V16[s][:, kb, lo:hi],
                                            start=False,
                                            stop=(s == 2 and kb == NB - 1),
                                        )
                        else:
                            nc.vector.memset(psi, 0.3)
                        evac(EV_RF[mb], rt[:, mb, :], psi, negate=False)
                        if not USE_LAYERNORM:
                            nc.vector.bn_stats(out=st6[:, mb, :],
                                               in_=rt[:, mb, ::STATS_STRIDE])

                    # plane-wide mean/var: per-partition bn stats aggregated
                    mv = spool.tile([P, 2], f32, tag="mv")
                    nc.vector.bn_aggr(out=mv, in_=st6)
                    t2 = spool.tile([P, 2], f16, tag="t2")
                    # E[x^2]_p = mean_p*mean_p + var_p, one fused op
                    nc.vector.scalar_tensor_tensor(
                        out=t2[:, 1:2], in0=mv[:, 0:1],
                        scalar=mv[:, 0:1], in1=mv[:, 1:2],
                        op0=mybir.AluOpType.mult,
                        op1=mybir.AluOpType.add)
                    nc.vector.tensor_copy(out=t2[:, 0:1], in_=mv[:, 0:1])
                    state2[p] = (rt, t2)

                def back_tail(p):
                    # deferred one pipeline step: by the time psS issues, the
                    # in-order PE stream has already run front(p+2)'s
                    # matmuls, so PE does not stall waiting for t2
                    rt, t2 = state2.pop(p)
                    yt = ypool.tile([P, NB, W], bf16, tag="y")
                    if "taildep" in ablate:
                        # timing probe: break Exp's dependency on the stats
                        # tail (constant rs/nbv written at plane start)
                        fin2 = spool.tile([P, 2], f32, tag="fin2")
                        nc.vector.memset(fin2, 1.0)
                    # ones-weights matmul sums across partitions AND
                    # broadcasts; out slot borrowed from the psi ring
                    # (same tag+shape: no extra PSUM bank).
                    psS = psip.tile([P, W], f32, tag="psi", name="psS")
                    ar = psS[:, 0:2]
                    nc.tensor.matmul(ar, ones16, t2, start=True, stop=True)

                    # short critical path to rs/nbv (every hop here delays
                    # the output Exp by an HW sem roundtrip):
                    #   fin01 = ar/P -> nvar = mean^2-E[x^2] (fused)
                    #   rs = exp(-0.5*ln(-c*nvar))  [ddof-1 c in the
                    #   negative Ln scale; drops the +eps (1e-8 rel on
                    #   std~1) and the reciprocal table op entirely]
                    fin = spool.tile([P, 4], f32, tag="fin")
                    mean = fin[:, 0:1]
                    nvar = fin[:, 1:2]
                    rs = fin[:, 2:3]
                    nbv = fin[:, 3:4]
                    nc.vector.tensor_scalar_mul(out=fin[:, 0:2],
                                                in0=ar[:, 0:2],
                                                scalar1=1.0 / P)
                    nc.vector.scalar_tensor_tensor(
                        out=nvar, in0=mean, scalar=mean, in1=fin[:, 1:2],
                        op0=mybir.AluOpType.mult,
                        op1=mybir.AluOpType.subtract)
                    npix = NPIX // STATS_STRIDE
                    nc.scalar.activation(out=rs, in_=nvar, func=AF.Ln,
                                         scale=-float(npix) / (npix - 1))
                    nc.scalar.activation(out=rs, in_=rs, func=AF.Exp,
                                         scale=-0.5)
                    # nbv = -mean * rs (one fused tensor_scalar: two ops)
                    nc.vector.tensor_scalar(out=nbv, in0=mean, scalar1=rs,
                                            scalar2=-1.0,
                                            op0=mybir.AluOpType.mult,
                                            op1=mybir.AluOpType.mult)
                    if "taildep" in ablate:
                        rs, nbv = fin2[:, 0:1], fin2[:, 1:2]
                    ydst = y[p].rearrange("(kb q) w -> q kb w", q=P)
                    if p == NPLANES - 1 and "act" not in ablate:
                        # last plane: two output chunks overlap Exp/min/DMA
                        # in the pipeline drain (one extra DMA's fixed cost
                        # buys ~half the Exp+DMA serialization back)
                        for hb in range(2):
                            sl = slice(2 * hb, 2 * hb + 2)
                            nc.scalar.activation(out=yt[:, sl, :],
                                                 in_=rt[:, sl, :],
                                                 func=AF.Exp,
                                                 bias=nbv, scale=rs)
                            nc.vector.tensor_scalar_min(
                                out=yt[:, sl, :], in0=yt[:, sl, :],
                                scalar1=1.0)
                            if "dma" not in ablate:
                                nc.sync.dma_start(out=ydst[:, sl, :],
                                                  in_=yt[:, sl, :])
                        return
                    if "act" in ablate:
                        nc.scalar.copy(out=yt, in_=rt)
                    else:
                        nc.scalar.activation(out=yt, in_=rt, func=AF.Exp,
                                             bias=nbv, scale=rs)
                    if MIN_ON_POOL:
                        nc.gpsimd.tensor_scalar_min(out=yt, in0=yt,
                                                    scalar1=1.0)
                    else:
                        nc.vector.tensor_scalar_min(out=yt, in0=yt,
                                                    scalar1=1.0)
                    if DEBUG_OUT == "refl":
                        nc.vector.tensor_copy(out=yt, in_=rt)
                    if "dma" not in ablate:
                        nc.sync.dma_start(out=ydst, in_=yt)

                # software-pipelined: pass 1 of plane p overlaps pass 2 of
                # p-1; the stats tail of p-2 slots between them
                for p in range(NPLANES + 2):
                    if p < NPLANES:
                        front(p)
                    if p >= 2:
                        back_tail(p - 2)
                    if 1 <= p <= NPLANES:
                        back(p - 1)

            if isinstance(reps, str) and reps.startswith("u"):
                for _ in range(int(reps[1:])):
                    emit_planes()
            elif reps == 1:
                emit_planes()
            else:
                from concourse import mybir as _mb
                with tc.For_i(0, reps, 1,
                              hint_engines=(_mb.EngineType.PE,)):
                    emit_planes()

    nc.compile()
    return nc


def get_program(reps=1):
    if reps not in _PROGRAM_CACHE:
        _PROGRAM_CACHE[reps] = build_program(reps)
    return _PROGRAM_CACHE[reps]


def _u_factors(k0, k1, k2):
    w = np.array([1.0, 0.75, 0.5], dtype=np.float64)
    w /= w.sum()
    us = []
    for s, k2d in enumerate((k0, k1, k2)):
        g = np.asarray(k2d)[0, 0].astype(np.float64)
        us.append((g.sum(axis=0), np.sqrt(w[s])))
    return us


def _toeplitz(u, scale, dtype, feedback=False):
    """Banded symmetric Toeplitz of scale*u. With feedback=True, quantize
    with per-column running error compensation (kills net kernel-mass bias
    from coarse dtypes like fp8)."""
    c = len(u) // 2
    V64 = np.zeros((H, W), dtype=np.float64)
    for d in range(-c, c + 1):
        V64 += np.diag(np.full(H - abs(d), scale * u[c + d]), k=d)
    if not feedback:
        return V64.astype(dtype)
    Vq = np.zeros((H, W), dtype=dtype)
    for j in range(W):
        lo, hi = max(0, j - c), min(H, j + c + 1)
        carry = 0.0
        for i in range(lo, hi):
            q = np.asarray(V64[i, j] + carry, dtype=dtype)
            carry = (V64[i, j] + carry) - float(q)
            Vq[i, j] = q
    return Vq


def build_v_matrices(k0, k1, k2):
    """fp16 banded Toeplitz matrices sqrt(w_s) * toeplitz(u_s) from the
    reference's 2-D depthwise kernels (u_s = column sums of the normalized
    2-D kernel, exact by separability)."""
    return [_toeplitz(u, sc, np.float16) for u, sc in _u_factors(k0, k1, k2)]


def build_const_inputs(k0, k1, k2):
    """Per-core constant input tensors (same on every core)."""
    us = _u_factors(k0, k1, k2)
    m = {f"v{s}": _toeplitz(u, sc, np.float16)
         for s, (u, sc) in enumerate(us)}
    if FP8_PASS2:
        import ml_dtypes
        for i, s in enumerate((1, 2)):
            u, sc = us[s]
            m[f"vq{i}"] = _toeplitz(u, sc, ml_dtypes.float8_e4m3,
                                    feedback=True)
    m["ident"] = np.eye(P, dtype=np.float32)
    return m


def kernel(rgb_image, k0, k1, k2):
    from concourse.bass_utils import run_bass_kernel_spmd

    nc = get_program()
    consts = build_const_inputs(k0, k1, k2)
    xs = np.asarray(rgb_image, dtype=np.float16)
    B = xs.shape[0]
    per_core = B // N_CORES
    in_maps = []
    for c in range(N_CORES):
        m = {"x": np.ascontiguousarray(
            xs[c * per_core:(c + 1) * per_core].reshape(NPLANES, H, W))}
        m.update(consts)
        in_maps.append(m)
    res = run_bass_kernel_spmd(nc, in_maps, list(range(N_CORES)))
    out = np.empty((B, 3, H, W), dtype=np.float32)
    for c in range(N_CORES):
        out[c * per_core:(c + 1) * per_core] = (
            res.results[c]["y"].astype(np.float32).reshape(per_core, 3, H, W))
    return out
